# revision 65
# baseline (speedup 1.0000x reference)
"""BertAttention (T5-style relative-position bias) Trainium2 Bass kernel.

Strategy (8-way tensor parallel over heads, 2 heads/core), v3:
  - Host pre-transposes hidden -> hT [HID, B*S]; per-core w_qkv column slice
    [Q h0|Q h1|K h0|K h1|V h0|V h1] (Q pre-scaled by 1/sqrt(HD)) so the QKV
    projection produces feature-major qkvT; V is PE-transposed into [k, d]
    layout (bf16) with an appended ones column (softmax denominator).
  - K projection bias is dropped exactly (softmax shift invariance); V bias is
    folded into the dense bias on the host (bd' = b_v @ w_dense + b_dense).
  - T5 buckets saturate for |k-q| >= 91: every score tile gets a per-(head,
    side) constant added free via the exp activation's bias operand; only the
    non-constant diagonal band (<=308 of 1024 columns per in-band tile, plus
    the smaller constant side) pays an anti-diagonal (flip) bias matmul whose
    rhs is a Toeplitz DMA read of host-precomputed side-shifted bias tables.
  - Scores per (batch, head, 1024-wide q chunk): s^T[k, q] in PSUM [128,1024],
    exp without max-subtraction -> es bf16; PV accumulates ctx^T [65, 1024].
  - ctx normalized to bf16; per (batch, head) 256KB bf16 AllToAll reshards
    head-split -> token-split as soon as each head finishes, overlapping the
    remaining attention; dense (bf16 weights, row-permuted on host to match
    the A2A layout) is computed transposed so bd' is a per-partition bias.
  - Emission order QKV(b0), attn(b0), QKV(b1), attn(b1), dense(b0), dense(b1)
    keeps the in-order PE queue dense while collectives overlap attention.
"""
import sys
import math

sys.path.insert(0, "/opt/trn_rl_repo")

import numpy as np
import ml_dtypes

import concourse.bass as bass
import concourse.bacc as bacc
import concourse.tile as tile
import concourse.mybir as mybir
from concourse.bass_utils import run_bass_kernel_spmd
from concourse.masks import make_identity

F32 = mybir.dt.float32
F32R = mybir.dt.float32r
BF16 = mybir.dt.bfloat16
FP8 = mybir.dt.float8e4
FP8_NP = mybir.dt.np(mybir.dt.float8e4)
DR = mybir.MatmulPerfMode.DoubleRow
Exp = mybir.ActivationFunctionType.Exp
ADD = mybir.AluOpType.add
MULT = mybir.AluOpType.mult

B, S, HID = 2, 2048, 1024
NH, HD = 16, 64
NB, MAXD = 32, 128
N_CORES = 8
HPC = NH // N_CORES          # heads per core = 2
T = B * S                    # 4096 flat tokens
KTILES = S // 128            # 16 k tiles per batch
QCH = 2                      # 1024-wide q chunks per batch
TW = 4096                    # padded width of expanded bias table
HB = S // N_CORES            # 256 tokens per core per batch
SAT = 91                     # |k-q| >= SAT -> bucket saturates (31 pos / 15 neg)


def _bucket_map_rev():
    """rev[z] = bucket(2047 - z) for z in [0, 4094], T5 bidirectional buckets."""
    rel = (2047 - np.arange(TW - 1)).astype(np.int64)   # k - q
    nb = NB // 2                                        # 16
    base = np.where(rel > 0, nb, 0)
    r = np.abs(rel)
    max_exact = nb // 2                                 # 8
    is_small = r < max_exact
    tmp = np.log(np.maximum(r, 1).astype(np.float32) / np.float32(max_exact))
    large = tmp / np.float32(math.log(MAXD / max_exact)) * np.float32(nb - max_exact)
    large_i = max_exact + large.astype(np.int32)
    large_i = np.minimum(large_i, nb - 1)
    return (base + np.where(is_small, r, large_i)).astype(np.int32)  # [4095]


def _tile_plan(kt, qch):
    """(side, w_lo, w_hi) for a [128k x 1024q] score tile.

    side 0: activation adds c_pos, side 1: adds c_neg. [w_lo, w_hi) is the
    column range (may be empty) where bias - c_side is nonzero; it is covered
    by a flip-matmul against the side-shifted Toeplitz table.
    """
    q0 = qch * 1024
    k0 = kt * 128
    b_lo = max(k0 - (SAT - 1) - q0, 0)           # cols < b_lo: all k-q >= SAT
    b_hi = min(k0 + 127 + (SAT - 1) - q0 + 1, 1024)  # cols >= b_hi: all k-q <= -SAT
    if b_lo >= 1024:       # fully pos-saturated
        return 0, 0, 0
    if b_hi <= 0:          # fully neg-saturated
        return 1, 0, 0
    left, right = b_lo, 1024 - b_hi
    if left >= right:      # leave left (pos side) to the constant
        return 0, b_lo, 1024
    return 1, 0, b_hi


def _build_program():
    nc = bacc.Bacc("TRN2", target_bir_lowering=False, debug=False,
                   enable_asserts=True, num_devices=N_CORES)

    hT_d = nc.dram_tensor("hT", [HID, T], F32R, kind="ExternalInput")
    wq_d = nc.dram_tensor("wq", [HID, 384], F32R, kind="ExternalInput")
    bq_d = nc.dram_tensor("bq", [128, 1], F32, kind="ExternalInput")
    wd_d = nc.dram_tensor("wd", [HID, HID], BF16, kind="ExternalInput")
    bd_d = nc.dram_tensor("bd", [HID, 1], F32, kind="ExternalInput")
    cv_d = nc.dram_tensor("cv", [128, 4], F32, kind="ExternalInput")
    tv_d = nc.dram_tensor("tv", [HPC, 2, TW], FP8, kind="ExternalInput")
    jp_d = nc.dram_tensor("jp", [64, 2, 128], FP8, kind="ExternalInput")
    out_d = nc.dram_tensor("outT", [HID, T // N_CORES], F32, kind="ExternalOutput")

    with tile.TileContext(nc) as tc:
        with tc.tile_pool(name="const", bufs=1) as cst, \
             tc.tile_pool(name="big", bufs=1) as big, \
             tc.tile_pool(name="dram", bufs=1, space="DRAM") as dram:

            # ---------------- constants / weights ----------------
            wq_sb = cst.tile([128, 8, 384], F32R, tag="wq")
            bq_sb = cst.tile([128, 1], F32, tag="bq")
            bd_sb = cst.tile([128, 8, 1], F32, tag="bd")
            cv_sb = cst.tile([128, 4], F32, tag="cv")
            wd_sb = big.tile([128, 8, HID], BF16, tag="wd")
            nc.gpsimd.dma_start(wd_sb[:], wd_d[:, :].rearrange("(j p) e -> p j e", p=128))
            ident_f = cst.tile([128, 128], F32, tag="identf")
            make_identity(nc, ident_f[:])
            ones64 = cst.tile([1, 64], F32, tag="ones64")
            nc.gpsimd.memset(ones64[:], 1.0)
            jp_sb = cst.tile([64, 2, 128], FP8, tag="jp")
            nc.sync.dma_start(jp_sb[:], jp_d[:, :, :])

            # packed fp8 Q/K for DoubleRow QK: [64 = 2 heads x 32, 2 k-subtiles, S]
            QPb = [big.tile([64, 2, S], FP8, tag=f"QP{b}", name=f"QP{b}") for b in range(B)]
            KPb = [big.tile([64, 2, S], FP8, tag=f"KP{b}", name=f"KP{b}") for b in range(B)]
            Vaugb = [big.tile([128, KTILES, 130], BF16, tag=f"Vaug{b}", name=f"Vaug{b}")
                     for b in range(B)]
            ctxb = [big.tile([128, S], BF16, tag=f"ctx{b}", name=f"ctx{b}")
                    for b in range(B)]

            a2a_in = [[dram.tile([512, HB], BF16, name=f"a2ai{b}_{s}")
                       for s in range(HPC)] for b in range(B)]
            a2a_out = [[dram.tile([512, HB], BF16, name=f"a2ao{b}_{s}")
                        for s in range(HPC)] for b in range(B)]

            tvb = tv_d[:, :, :]

            with tc.tile_pool(name="htp", bufs=8) as htp, \
                 tc.tile_pool(name="qkst", bufs=4) as qkst, \
                 tc.tile_pool(name="vtp", bufs=2) as vtp, \
                 tc.tile_pool(name="rp", bufs=14) as rp, \
                 tc.tile_pool(name="expp", bufs=6) as expp, \
                 tc.tile_pool(name="nrm", bufs=4) as nrm, \
                 tc.tile_pool(name="sps", bufs=3, space="PSUM") as sps, \
                 tc.tile_pool(name="cps", bufs=1, space="PSUM") as cps:

                qkv_state = {}
                pending_shuf = []

                def flush_shuf():
                    while pending_shuf:
                        pending_shuf.pop(0)()

                def qkv_chunk_pieces(b, tci, eager_eng=None):
                    """Emission pieces (~1.7us PE each) so projection work can
                    weave between attention kt tiles without draining the
                    ACT exp buffer."""
                    state = {}

                    def piece_a():
                        if tci == 0:
                            qkv_state[b] = (
                                vtp.tile([128, S], F32, tag="VT", name=f"VT{b}"),
                                qkst.tile([128, S], FP8, tag="qst", name=f"qst{b}"),
                                qkst.tile([128, S], FP8, tag="kst", name=f"kst{b}"))
                            nc.gpsimd.memset(
                                Vaugb[b][:].rearrange("p t (g c) -> p t g c", c=65)
                                [:, :, :, 64:65], 1.0)
                        VT, qstF, kstF = qkv_state[b]
                        gci = b * 4 + tci
                        htt = []
                        first = b == 0 and tci == 0
                        for kd in range(4):
                            if first:
                                nc.sync.dma_start(wq_sb[:, 2 * kd, :],
                                                  wq_d[256 * kd:256 * kd + 128, :])
                                nc.sync.dma_start(wq_sb[:, 2 * kd + 1, :],
                                                  wq_d[256 * kd + 128:256 * kd + 256, :])
                            ht = htp.tile([128, 1024], F32R, tag="ht")
                            eng = nc.sync if (kd % 2 == 0 or b == 1) else nc.scalar
                            eng.dma_start(
                                ht[:].rearrange("p (kd t) -> p kd t", t=512),
                                hT_d[:, 512 * gci:512 * (gci + 1)]
                                .rearrange("(kd p) t -> p kd t", p=128)
                                [:, 2 * kd:2 * kd + 2, :])
                            htt.append(ht)
                        flush_shuf()
                        state["hts"] = [
                            htt[kt // 2][:, 512 * (kt % 2):512 * (kt % 2 + 1)]
                            for kt in range(8)]
                        if first:
                            nc.sync.dma_start(bq_sb[:], bq_d[:, :])
                            nc.sync.dma_start(cv_sb[:], cv_d[:, :])
                            nc.sync.dma_start(
                                bd_sb[:], bd_d[:, :].rearrange("(e p) o -> p e o", p=128))
                        pa = sps.tile([128, 1024], F32, tag="S")
                        state["pa"] = pa
                        for kt in range(8):
                            nc.tensor.matmul(pa[:, 0:512], wq_sb[:, kt, 0:128],
                                             state["hts"][kt],
                                             start=(kt == 0), stop=(kt == 7))

                    def piece_b():
                        VT, qstF, kstF = qkv_state[b]
                        pa = state["pa"]
                        for kt in range(8):
                            nc.tensor.matmul(pa[:, 512:1024], wq_sb[:, kt, 128:256],
                                             state["hts"][kt],
                                             start=(kt == 0), stop=(kt == 7))
                        nc.vector.tensor_tensor(
                            qstF[:, 512 * tci:512 * (tci + 1)], pa[:, 0:512],
                            bq_sb[:, 0:1].to_broadcast([128, 512]), ADD)
                        nc.vector.tensor_copy(
                            kstF[:, 512 * tci:512 * (tci + 1)], pa[:, 512:1024])
                        cols = slice(512 * tci, 512 * (tci + 1))

                        def do_shuf(b=b, cols=cols, qstF=qstF, kstF=kstF):
                            eng = eager_eng or (nc.scalar if b == 0 else nc.gpsimd)
                            for s in range(HPC):
                                for i in range(2):
                                    o = 64 * s + 32 * i
                                    eng.dma_start(
                                        QPb[b][32 * s:32 * s + 32, i, cols],
                                        qstF[o:o + 32, cols])
                                    eng.dma_start(
                                        KPb[b][32 * s:32 * s + 32, i, cols],
                                        kstF[o:o + 32, cols])
                        if eager_eng is not None:
                            do_shuf()
                        else:
                            pending_shuf.append(do_shuf)

                    def piece_c():
                        VT, qstF, kstF = qkv_state[b]
                        pb = sps.tile([128, 1024], F32, tag="S")
                        state["pb"] = pb
                        for kt in range(8):
                            nc.tensor.matmul(pb[:, 0:512], wq_sb[:, kt, 256:384],
                                             state["hts"][kt],
                                             start=(kt == 0), stop=(kt == 7))
                        nc.vector.tensor_copy(
                            VT[:, 512 * tci:512 * (tci + 1)], pb[:, 0:512])

                    def piece_d():
                        VT, qstF, kstF = qkv_state[b]
                        pb = state["pb"]
                        for t4 in range(4):
                            t = 4 * tci + t4
                            vslot = Vaugb[b][:, t, :].rearrange("p (g c) -> p g c", c=65)
                            tp = pb[:, 512 + 128 * t4:640 + 128 * t4]
                            nc.tensor.transpose(tp, VT[:, 128 * t:128 * (t + 1)],
                                                ident_f[:])
                            nc.vector.tensor_copy(
                                vslot[:, :, 0:64],
                                tp.rearrange("p (g c) -> p g c", c=64))
                        flush_shuf()

                    return [piece_a, piece_b, piece_c, piece_d]

                def qkv_chunk(b, tci):
                    for p in qkv_chunk_pieces(b, tci):
                        p()

                def attn_phase(b, piece_filler=None):
                    for s in range(HPC):
                        for qch in range(QCH):
                            pf = (piece_filler(2 * s + qch)
                                  if piece_filler is not None else None)
                            points, pieces = pf if pf else ((), [])
                            q0 = qch * 1024
                            plans = [_tile_plan(kt, qch) for kt in range(KTILES)]
                            rts = {}
                            for kt in range(KTILES):
                                side, w_lo, w_hi = plans[kt]
                                if w_hi <= w_lo:
                                    continue
                                w = w_hi - w_lo
                                r = rp.tile([64, 2, 768], FP8, tag="rt")
                                src = bass.AP(
                                    tvb.tensor,
                                    tvb.offset + (2 * s + side) * TW
                                    + (1920 - kt * 128 + q0 + w_lo),
                                    [[1, 64], [64, 2], [1, w]])
                                nc.sync.dma_start(r[:, :, 0:w], src)
                                rts[kt] = r
                            ctx_ps = cps.tile([65, 1024], F32, tag="ctx",
                                              name=f"ctx{b}_{s}_{qch}")
                            def emit_pv(kt, es):
                                for eh in range(2):
                                    nc.tensor.matmul(ctx_ps[:, 512 * eh:512 * (eh + 1)],
                                                     Vaugb[b][:, kt, 65 * s:65 * s + 65],
                                                     es[:, 512 * eh:512 * (eh + 1)],
                                                     start=(kt == 0),
                                                     stop=(kt == KTILES - 1))

                            # PV lags QK/exp by 3 tiles so the first PV of a
                            # unit (carrying the anti-dep on the single ctx
                            # PSUM buffer) never stalls the in-order PE queue.
                            pvq = []
                            for kt in range(KTILES):
                                side, w_lo, w_hi = plans[kt]
                                s_ps = sps.tile([128, 1024], F32, tag="S")
                                for qh in range(2):
                                    r0, r1 = 512 * qh, 512 * (qh + 1)
                                    has_bias = w_lo < r1 and w_hi > r0
                                    nc.tensor.matmul(
                                        s_ps[:, r0:r1],
                                        KPb[b][32 * s:32 * s + 32, :,
                                               kt * 128:kt * 128 + 128],
                                        QPb[b][32 * s:32 * s + 32, :,
                                               q0 + r0:q0 + r1],
                                        start=True, stop=not has_bias,
                                        perf_mode=DR)
                                    if has_bias:
                                        c0, c1 = max(w_lo, r0), min(w_hi, r1)
                                        nc.tensor.matmul(
                                            s_ps[:, c0:c1], jp_sb[:],
                                            rts[kt][:, :, c0 - w_lo:c1 - w_lo],
                                            start=False, stop=True, perf_mode=DR)
                                es = expp.tile([128, 1024], BF16, tag="es")
                                nc.scalar.activation(
                                    es[:], s_ps[:], Exp,
                                    bias=cv_sb[:, 2 * s + side:2 * s + side + 1])
                                pvq.append((kt, es))
                                if len(pvq) > 3:
                                    emit_pv(*pvq.pop(0))
                                if kt in points and pieces:
                                    pieces.pop(0)()
                            while pvq:
                                emit_pv(*pvq.pop(0))
                            cc = nrm.tile([65, 1024], F32, tag="cc")
                            nc.vector.tensor_copy(cc[:], ctx_ps[:])
                            recip = nrm.tile([1, 1024], F32, tag="rc")
                            nc.vector.reciprocal(recip[:], cc[64:65, :])
                            rbb = nrm.tile([64, 1024], F32, tag="rb")
                            nc.gpsimd.partition_broadcast(rbb[:], recip[:])
                            nc.vector.tensor_tensor(
                                ctxb[b][64 * s:64 * s + 64, q0:q0 + 1024],
                                cc[0:64, :], rbb[:], MULT)
                            nc.gpsimd.dma_start(
                                a2a_in[b][s][256 * qch:256 * (qch + 1), :]
                                .rearrange("(j d) t -> d j t", d=64),
                                ctxb[b][64 * s:64 * s + 64, q0:q0 + 1024]
                                .rearrange("d (j t) -> d j t", t=HB))

                    # head (b, s) staged: all-to-all overlaps the rest
                        nc.gpsimd.collective_compute(
                            "AllToAll", mybir.AluOpType.bypass,
                            replica_groups=[list(range(N_CORES))],
                            ins=[a2a_in[b][s][:].opt()],
                            outs=[a2a_out[b][s][:].opt()])

                for tci in range(3):
                    for p in qkv_chunk_pieces(0, tci, eager_eng=nc.gpsimd):
                        p()

                def f0(u):
                    if u == 0:
                        return ((2, 4, 6, 8),
                                qkv_chunk_pieces(0, 3, eager_eng=nc.gpsimd))
                    return ((2, 6, 10, 14), qkv_chunk_pieces(1, u - 1))

                def f1(u):
                    if u == 0:
                        return ((2, 4, 6, 8),
                                qkv_chunk_pieces(1, 3, eager_eng=nc.sync))
                    return None

                attn_phase(0, piece_filler=f0)
                flush_shuf()
                attn_phase(1, piece_filler=f1)

            # ---------------- dense (after A2A) ----------------
            with tc.tile_pool(name="dns", bufs=8) as dns, \
                 tc.tile_pool(name="dno", bufs=1) as dno, \
                 tc.tile_pool(name="dps", bufs=8, space="PSUM") as dps:
                outT_sb = dno.tile([128, 8, 2 * HB], F32, tag="outT")
                out_r = out_d[:, :].rearrange("(e p) t -> p e t", p=128)
                for b in range(B):
                    psb = [dps.tile([128, HB], F32, tag="d", name=f"d{b}_{e}")
                           for e in range(8)]
                    for blk in range(8):        # blk = s*4 + j
                        s, j = blk // 4, blk % 4
                        cf = dns.tile([128, HB], BF16, tag="cf")
                        ceng = nc.sync if blk % 2 == 0 else nc.scalar
                        ceng.dma_start(cf[:], a2a_out[b][s][128 * j:128 * (j + 1), :])
                        for e in range(8):
                            nc.tensor.matmul(psb[e][:],
                                             wd_sb[:, blk, 128 * e:128 * (e + 1)],
                                             cf[:], start=(blk == 0), stop=(blk == 7))
                    for e in range(8):
                        nc.vector.tensor_tensor(
                            outT_sb[:, e, b * HB:(b + 1) * HB], psb[e][:],
                            bd_sb[:, e, 0:1].to_broadcast([128, HB]), ADD)
                        deng = nc.sync if e % 2 == 0 else nc.scalar
                        deng.dma_start(
                            out_r[:, e, b * HB:(b + 1) * HB],
                            outT_sb[:, e, b * HB:(b + 1) * HB])

    nc.compile()
    return nc


_NC_CACHE = None


def _get_program():
    global _NC_CACHE
    if _NC_CACHE is None:
        _NC_CACHE = _build_program()
    return _NC_CACHE


def _make_inmaps(hidden_states, w_qkv, b_qkv, w_dense, b_dense, rel_attn_table):
    hidden_states = np.asarray(hidden_states, dtype=np.float32)
    w_qkv = np.asarray(w_qkv, dtype=np.float32)
    b_qkv = np.asarray(b_qkv, dtype=np.float32)
    w_dense = np.asarray(w_dense, dtype=np.float32)
    b_dense = np.asarray(b_dense, dtype=np.float32)
    rel_attn_table = np.asarray(rel_attn_table, dtype=np.float32)

    hT = np.ascontiguousarray(hidden_states.reshape(T, HID).T)   # [HID, T]
    bm = _bucket_map_rev()
    scale = np.float32(1.0 / math.sqrt(HD))
    # packed anti-diagonal (flip) matrix for DoubleRow bias matmuls:
    # jp[k', i, m] = 1 iff 64*i + k' == 127 - m
    jp_pack = np.zeros((64, 2, 128), dtype=FP8_NP)
    for i in range(2):
        for kp in range(64):
            jp_pack[kp, i, 127 - 64 * i - kp] = FP8_NP(1.0)

    # dense weights row-permuted to the A2A row order (s, c, d) and bf16;
    # V projection bias folded into the dense bias (exact)
    wd_perm = np.ascontiguousarray(
        w_dense.reshape(N_CORES, HPC, HD, HID).transpose(1, 0, 2, 3)
        .reshape(HID, HID)).astype(ml_dtypes.bfloat16)
    bv = b_qkv[2 * HID:3 * HID]
    bd_f = (bv @ w_dense + b_dense).astype(np.float32).reshape(HID, 1)

    in_maps = []
    for c in range(N_CORES):
        ha, hb = HPC * c, HPC * c + 1
        qcols = [w_qkv[:, h * HD:(h + 1) * HD] * scale for h in (ha, hb)]
        kcols = [w_qkv[:, HID + h * HD:HID + (h + 1) * HD] for h in (ha, hb)]
        vcols = [w_qkv[:, 2 * HID + h * HD:2 * HID + (h + 1) * HD] for h in (ha, hb)]
        wq_c = np.ascontiguousarray(
            np.concatenate(qcols + kcols + vcols, axis=1))              # [HID, 384]
        bq_c = np.concatenate(
            [b_qkv[h * HD:(h + 1) * HD] * scale for h in (ha, hb)]
        ).reshape(128, 1).astype(np.float32)
        # expanded bias table minus the side constant: [head slot, side, TW]
        texp = rel_attn_table[[ha, hb]][:, bm]                          # [2, 4095]
        tv_c = np.zeros((HPC, 2, TW), dtype=FP8_NP)
        for si, bucket in ((0, 31), (1, 15)):
            tv_c[:, si, :TW - 1] = (
                texp - rel_attn_table[[ha, hb], bucket][:, None]
            ).astype(FP8_NP)
        cv_c = np.tile(
            np.array([rel_attn_table[ha, 31], rel_attn_table[ha, 15],
                      rel_attn_table[hb, 31], rel_attn_table[hb, 15]],
                     dtype=np.float32), (128, 1))
        in_maps.append({
            "hT": hT,
            "wq": wq_c,
            "bq": bq_c,
            "wd": wd_perm,
            "bd": bd_f,
            "cv": cv_c,
            "tv": tv_c,
            "jp": jp_pack,
        })
    return in_maps


def kernel(hidden_states, w_qkv, b_qkv, w_dense, b_dense, rel_attn_table):
    in_maps = _make_inmaps(hidden_states, w_qkv, b_qkv, w_dense, b_dense,
                           rel_attn_table)
    nc = _get_program()
    res = run_bass_kernel_spmd(nc, in_maps, core_ids=list(range(N_CORES)))
    full = np.empty((HID, T), dtype=np.float32)
    for c in range(N_CORES):
        o = res.results[c]["outT"]            # [HID, 2*HB]: [b0 block c | b1 block c]
        full[:, c * HB:(c + 1) * HB] = o[:, :HB]
        full[:, S + c * HB:S + (c + 1) * HB] = o[:, HB:]
    return np.ascontiguousarray(full.T).reshape(B, S, HID)
